# revision 1
# baseline (speedup 1.0000x reference)
"""CAML-style multi-label attention kernel for Trainium2 (8 NeuronCores).

Reference computation (B=8, W=1000, V=50000, E=100, C=50, K=3, L=18000):
    emb    = W_embed[x]                            (B, W, E)
    H      = tanh(conv1d(emb, conv_w) + conv_b)    (B, W, C)  'same' padding
    scores = einsum("lc,bwc->blw", u_w, H)
    attns  = softmax(scores, axis=w)
    m      = einsum("blw,bwc->blc", attns, H)
    out    = sigmoid(sum(out_w * m, axis=c) + out_b)   (B, L)

Sharding: L=18000 split across 8 cores (2250 labels each, padded to 2304).
The (tiny) conv prologue is replicated on every core.

Per-core algorithm (fp8e4m3 fast path; rel-err budget is 2e-2, this achieves
~4e-4 -- scores/weights are O(0.6) here so e4m3 quantization washes out
through the softmax):
  - host pre-gathers embedding rows into conv-rhs layout (embt, one DMA);
    conv = 3 accumulating bf16 matmuls; tanh writes H in fp8e4m3.
  - mm1 (scores): plain fp8 matmul, H stationary, u^T fp8 streaming.
  - exp via the Schraudolph bit trick: int8(SCHR_A*s + SCHR_B) reinterpreted
    as fp8e4m3 IS exp(s) (~7% pointwise, ~4e-4 after softmax averaging).
    A plain affine+convert, so it runs on BOTH ScalarE (activation Copy with
    scale/bias) and VectorE (tensor_scalar), splitting the 21M-element
    PSUM->SBUF exp pass across two engines. (GpSimd cannot access PSUM.)
  - mm2 (pooling): fp8 DoubleRow pairing adjacent 128-token chunks: lhsT is
    Haug8 [w128, (2, 64)] = H^T + ones column (-> softmax denominator) + pad
    (DoubleRow needs block stride %16==0); rhs is the ex tile [w128, (2,LW)]
    whose halves the exp stage already writes. fp8 PE transposes need
    element-step-2 output APs.
  - ma (PSUM) staged to SBUF bf16 (ScalarE/VectorE alternating), small PE
    transposes to label-partition layout, epilogue dot/divide on
    VectorE/GpSimd, final sigmoid on ScalarE.
  - per-batch prologue is software-pipelined one batch ahead of its main
    loop; DMA count per iteration is minimized (hoisted input loads, one
    batched output store) -- each in-body DMA costs several us of per-launch
    ring overhead on this hardware, dwarfing its nominal transfer time.
"""

import os

import numpy as np

try:
    import concourse.bass as bass
except ImportError:  # repo not on sys.path in fresh dirs
    import sys

    sys.path.insert(0, "/opt/trn_rl_repo")
    import concourse.bass as bass

import concourse.bacc as bacc
import concourse.tile as tile
from concourse import mybir
from concourse.bass import IndirectOffsetOnAxis
from concourse.bass_utils import run_bass_kernel_spmd
from concourse.masks import make_identity

FP = mybir.dt.float32
BF = mybir.dt.bfloat16
F8 = mybir.dt.float8e4
I8 = mybir.dt.int8
U8 = mybir.dt.uint8
AF = mybir.ActivationFunctionType
ALU = mybir.AluOpType
DR = mybir.MatmulPerfMode.DoubleRow

B, W, V, E, C, K, L = 8, 1000, 50000, 100, 50, 3, 18000
NCORES = 8
WPAD = 1024  # W padded to 8 chunks of 128
LSH = L // NCORES  # 2250 labels per core
LPAD = 2304  # 18 tiles of 128
LT = LPAD // 128  # 18 label tiles per core
NCI = WPAD // 128  # 8 w-chunks
CH = C // 2  # 25: contraction rows for DoubleRow mm1
C2 = 64  # Haug block width: H cols + ones col + zero pad (DoubleRow needs block stride %16==0)

# Schraudolph exp in fp8e4m3: exp(x) ~= bitcast_f8(int8(SCHR_A*x + SCHR_B))
SCHR_A = float(os.environ.get("SCHR_A", 8.0 / np.log(2.0)))
SCHR_B = float(os.environ.get("SCHR_B", "55.85"))

# exp engine split: counts out of 20 pair-chunks ([128, 1024]) per batch.
# GPSIMD cannot access PSUM on TRN2, so exp runs on ScalarE + VectorE only.
EXP_NA = int(os.environ.get("EXP_NA", "11"))  # ScalarE (activation Copy)
EXP_ND = int(os.environ.get("EXP_ND", "9"))  # VectorE (rest)

# ma PSUM->SBUF staging engine per lb-block (5 chars, A/D/P)
MSB_ENG = os.environ.get("MSB_ENG", "ADADA")

# 1: host pre-gathers embedding rows (embx input, DMA on SP queue);
# 0: device-side indirect gather on GpSimd
EMB_HOST = int(os.environ.get("EMB_HOST", "1"))

EXP_GRAN = os.environ.get("EXP_GRAN", "half")  # "pair" or "half"
# HW ablation: 5=full, 4=no epilogue, 3=no post(msb/ptm/mlt), 2=no mm2,
# 1=no exp, 0=prologue only
STAGE = int(os.environ.get("STAGE", "5"))
EXPP_BUFS = int(os.environ.get("EXPP_BUFS", "4"))
PSA_BUFS = int(os.environ.get("PSA_BUFS", "4"))
PSB_BUFS = int(os.environ.get("PSB_BUFS", "2"))
PST_BUFS = int(os.environ.get("PST_BUFS", "1"))


def _mix_assign(na, nd, np_, total=40):
    """Interleave engine assignments smoothly (weighted round-robin)."""
    quota = {"A": na, "D": nd, "P": np_}
    acc = {"A": 0.0, "D": 0.0, "P": 0.0}
    out = []
    for _ in range(total):
        for k in acc:
            acc[k] += quota[k] / total
        k = max(acc, key=lambda q: acc[q])
        acc[k] -= 1.0
        out.append(k)
    return out


def build_nc(num_devices: int, repeat: int = 1):
    nc = bacc.Bacc(
        "TRN2", target_bir_lowering=False, debug=False, num_devices=num_devices
    )

    if EMB_HOST:
        wemb = nc.dram_tensor("embt", [E, B * 1032], BF, kind="ExternalInput").ap()
    else:
        x_idx = nc.dram_tensor("x_idx", [128, B * NCI], mybir.dt.int32, kind="ExternalInput").ap()
        wemb = nc.dram_tensor("wemb", [V, E], FP, kind="ExternalInput").ap()
    convwt = nc.dram_tensor("convwt", [E, K * C], BF, kind="ExternalInput").ap()
    convb = nc.dram_tensor("convb", [C, 1], FP, kind="ExternalInput").ap()
    uwt8 = nc.dram_tensor("uwt8", [64, LPAD], U8, kind="ExternalInput").ap()
    owp = nc.dram_tensor("owp", [128, LT * C], BF, kind="ExternalInput").ap()
    obp = nc.dram_tensor("obp", [128, LT], FP, kind="ExternalInput").ap()
    out = nc.dram_tensor("out", [B, 128, LT], FP, kind="ExternalOutput").ap()

    with tile.TileContext(nc) as tc:
        with tc.tile_pool(name="const", bufs=1) as constp:
            # all input loads happen ONCE per launch, outside the repeat
            # body (each DMA in the body costs ~7.5us of per-launch ring
            # overhead on HW, dwarfing the compute)
            cst = {}
            cst["ident"] = constp.tile([128, 128], FP, name="ident")
            make_identity(nc, cst["ident"])
            cst["ident_bf"] = constp.tile([128, 128], BF, name="ident_bf")
            make_identity(nc, cst["ident_bf"])
            cst["ident_f8"] = constp.tile([128, 128], F8, name="ident_f8")
            make_identity(nc, cst["ident_f8"])
            uwt8_s = constp.tile([64, LPAD], U8, name="uwt8_s")
            nc.sync.dma_start(out=uwt8_s, in_=uwt8)
            cst["uwt8_v"] = uwt8_s.bitcast(F8)
            cst["convwt_s"] = constp.tile([E, K * C], BF, name="convwt_s")
            nc.sync.dma_start(out=cst["convwt_s"], in_=convwt)
            cst["convb_s"] = constp.tile([C, 1], FP, name="convb_s")
            nc.sync.dma_start(out=cst["convb_s"], in_=convb)
            cst["owp_s"] = constp.tile([128, LT * C], BF, name="owp_s")
            nc.sync.dma_start(out=cst["owp_s"], in_=owp)
            cst["obp_s"] = constp.tile([128, LT], FP, name="obp_s")
            nc.sync.dma_start(out=cst["obp_s"], in_=obp)
            if EMB_HOST:
                cst["embt_s"] = constp.tile([E, B * 1032], BF, name="embt_s")
                nc.sync.dma_start(out=cst["embt_s"], in_=wemb)
                cst["idx_s"] = None
            else:
                cst["idx_s"] = constp.tile(
                    [128, B * NCI], mybir.dt.int32, name="idx_s"
                )
                nc.sync.dma_start(out=cst["idx_s"], in_=x_idx)
                cst["wemb"] = wemb
            for _ in range(repeat):
                _body(tc, nc, cst, out)
    nc.compile()
    return nc


def _body(tc, nc, cst, out):
    assign = _mix_assign(EXP_NA, EXP_ND, 20 - EXP_NA - EXP_ND, total=20)
    ident = cst["ident"]
    ident_bf = cst["ident_bf"]
    ident_f8 = cst["ident_f8"]
    uwt8_v = cst["uwt8_v"]
    convwt_s = cst["convwt_s"]
    convb_s = cst["convb_s"]
    owp_s = cst["owp_s"]
    obp_s = cst["obp_s"]
    with (
        tc.tile_pool(name="work", bufs=2) as workp,
        tc.tile_pool(name="expp", bufs=EXPP_BUFS) as expp,
        tc.tile_pool(name="psA", bufs=PSA_BUFS, space="PSUM") as psA,  # sc
        tc.tile_pool(name="psB", bufs=PSB_BUFS, space="PSUM") as psB,  # ma
        tc.tile_pool(name="psT", bufs=PST_BUFS, space="PSUM") as psT,  # ptm
        tc.tile_pool(name="psP", bufs=1, space="PSUM") as psP,  # prologue
        tc.tile_pool(name="outp", bufs=2) as outp,
    ):
        def gather_emb(b):
            # non-host path: 8 indirect gathers on the GpSimd SWDGE queue
            embG = workp.tile([128, NCI * E], BF, tag="embG", name="embG", bufs=3)
            for ci in range(NCI):
                nc.gpsimd.indirect_dma_start(
                    out=embG[:, ci * E : (ci + 1) * E],
                    out_offset=None,
                    in_=cst["wemb"][:, :],
                    in_offset=IndirectOffsetOnAxis(
                        ap=cst["idx_s"][:, b * NCI + ci : b * NCI + ci + 1], axis=0
                    ),
                )
            return embG

        def prologue(b, embG_b):
            # embP -> conv -> tanh -> H tiles (fp8). Issued one batch AHEAD
            # of its main loop so its engine slots sit in front of the
            # previous batch's exp backlog in the queues.
            Hf8 = workp.tile([C, WPAD], F8, tag="Hf8", name="Hf8")
            Haug8 = workp.tile([128, NCI * C2], F8, tag="Haug8", name="Haug8")
            if EMB_HOST:
                embP = cst["embt_s"][:, b * 1032 : (b + 1) * 1032]
            else:
                embP = workp.tile([E, 1032], BF, tag="embP", name="embP")
                nc.gpsimd.memset(embP[:, 0:1], 0.0)
                nc.gpsimd.memset(embP[:, 1001:1032], 0.0)
                for ci in range(NCI):
                    pt = psP.tile([128, 128], BF, tag="pp", name="pt")
                    nc.tensor.transpose(
                        out=pt[:E, :],
                        in_=embG_b[:, ci * E : (ci + 1) * E],
                        identity=ident_bf[:, :],
                    )
                    cw = min(128, W - ci * 128)
                    nc.vector.tensor_copy(
                        out=embP[:, 1 + ci * 128 : 1 + ci * 128 + cw], in_=pt[:E, :cw]
                    )

            # conv1d: H[c, w] = tanh(sum_k convw_k.T @ embP[:, w+k] + b)
            for w0, cw in ((0, 512), (512, W - 512)):
                pm = psP.tile([C, 512], FP, tag="pp", name="convps")
                for k in range(K):
                    nc.tensor.matmul(
                        out=pm[:C, :cw],
                        lhsT=convwt_s[:, k * C : (k + 1) * C],
                        rhs=embP[:, w0 + k : w0 + k + cw],
                        start=(k == 0),
                        stop=(k == K - 1),
                    )
                nc.scalar.activation(
                    out=Hf8[:C, w0 : w0 + cw],
                    in_=pm[:C, :cw],
                    func=AF.Tanh,
                    bias=convb_s[:, 0:1],
                )
            nc.gpsimd.memset(Hf8[:C, W:WPAD], 0.0)

            # Haug8[w, ci*C2 + (0:C)] = H^T chunk; col C = 1.0 (0 on pads)
            for ci in range(NCI):
                pt2 = psP.tile([128, 256], F8, tag="pp", name="pt2")
                # fp8 transpose requires output element step 2 (16-bit PE
                # datapath); write strided, then the pack copy reads strided
                pt2v = pt2.rearrange("p (c two) -> p two c", two=2)
                nc.tensor.transpose(
                    out=pt2v[:, 0, :C],
                    in_=Hf8[:C, ci * 128 : (ci + 1) * 128],
                    identity=ident_f8[:C, :C],
                )
                base = ci * C2
                nc.vector.tensor_copy(
                    out=Haug8[:, base : base + C], in_=pt2v[:, 0, :C]
                )
                nc.gpsimd.memset(Haug8[:, base + C + 1 : base + C2], 0.0)
                if ci < NCI - 1:
                    nc.gpsimd.memset(Haug8[:, base + C : base + C + 1], 1.0)
                else:
                    nc.gpsimd.memset(Haug8[:, base + C : base + C + 1], 0.0)
                    nc.gpsimd.memset(Haug8[: W - 896, base + C : base + C + 1], 1.0)
            return Hf8, Haug8

        # batched output staging: one DMA per body instead of one per batch
        osball = outp.tile([128, B * LT], FP, tag="osball", name="osball")

        # pipeline fill: prologue leads the main loop by 1 batch
        if EMB_HOST:
            H_cur = prologue(0, None)
            embG_next = None
        else:
            embG_next = gather_emb(0)
            H_cur = prologue(0, embG_next)
            if B > 1:
                embG_next = gather_emb(1)

        for b in range(B):
            Hf8, Haug8 = H_cur
            if b + 1 < B:
                H_cur = prologue(b + 1, embG_next)
            if not EMB_HOST and b + 2 < B:
                embG_next = gather_emb(b + 2)

            # ------------- main: label blocks for this batch ----------------
            # per-batch label-partition results: [p, lt*64 + (0..49 m, 50 s)]
            mlt = workp.tile([128, LT * 64], BF, tag="mlt", name="mlt")

            def lb_post(ma_u, lb_u, LW_u):
                # PSUM -> SBUF staging (to bf16, engine tunable), then small
                # PE transposes back to label-partition layout, packed
                # 4-per-bank so one strided 2x-mode copy moves them all
                msb = workp.tile([C + 1, 512], BF, tag="msb", name="msb")
                meng = MSB_ENG[lb_u % len(MSB_ENG)]
                if meng == "A":
                    nc.scalar.activation(
                        out=msb[:, :LW_u], in_=ma_u[: C + 1, :LW_u], func=AF.Copy
                    )
                else:
                    nc.vector.tensor_copy(out=msb[:, :LW_u], in_=ma_u[: C + 1, :LW_u])
                nq = LW_u // 128
                ptm = psT.tile([128, 256], BF, tag="pt", name="ptm")
                for q in range(nq):
                    nc.tensor.transpose(
                        out=ptm[:, q * 64 : q * 64 + C + 1],
                        in_=msb[:, q * 128 : (q + 1) * 128],
                        identity=ident_bf[: C + 1, : C + 1],
                    )
                nc.vector.tensor_copy(
                    out=mlt.rearrange("p (t s) -> p t s", s=64)[
                        :, lb_u * 4 : lb_u * 4 + nq, 0 : C + 1
                    ],
                    in_=ptm.rearrange("p (q s) -> p q s", s=64)[:, 0:nq, 0 : C + 1],
                )

            def emit_mm2(u):
                ma_u, ex_u, lb_u, LW_u, pair_u = u
                lhsT = Haug8[
                    :, pair_u * 2 * C2 : (pair_u * 2 + 2) * C2
                ].rearrange("p (two f) -> p two f", two=2)
                rhs = ex_u.bitcast(F8).rearrange("p (two l) -> p two l", two=2)[
                    :, :, :LW_u
                ]
                nc.tensor.matmul(
                    out=ma_u[:, :LW_u],
                    lhsT=lhsT,
                    rhs=rhs,
                    start=(pair_u == 0),
                    stop=(pair_u == 3),
                    perf_mode=DR,
                )
                if pair_u == 3 and STAGE >= 4:
                    lb_post(ma_u, lb_u, LW_u)

            def emit_exp(eng, src, dst):
                if eng == "A":
                    nc.scalar.activation(
                        out=dst, in_=src, func=AF.Copy,
                        bias=SCHR_B, scale=SCHR_A,
                    )
                else:
                    nc.vector.tensor_scalar(
                        out=dst, in0=src, scalar1=SCHR_A,
                        scalar2=SCHR_B, op0=ALU.mult, op1=ALU.add,
                    )

            # one-stage software pipeline: emit mm2 of the previous pair after
            # this pair's mm1+exp, so PE is never queued behind a stalled mm2
            pend = None
            eidx = 0
            for lb in range(5):
                lb0 = lb * 512
                LW = min(512, LPAD - lb0)
                ma = psB.tile([C2, 512], FP, tag="maug", name="ma")
                for pair in range(4):
                    ex = expp.tile([128, 1024], U8, tag="ex", name="ex")
                    exv = ex.bitcast(I8)
                    if EXP_GRAN == "pair":
                        sc = psA.tile([128, 1024], FP, tag="sc", name="sc")
                        for h in range(2):
                            ci = pair * 2 + h
                            if STAGE >= 1:
                                nc.tensor.matmul(
                                    out=sc[:, h * 512 : h * 512 + LW],
                                    lhsT=Hf8[:C, ci * 128 : (ci + 1) * 128],
                                    rhs=uwt8_v[:C, lb0 : lb0 + LW],
                                    start=True,
                                    stop=True,
                                )
                        # one Schraudolph-exp instruction per pair (strided
                        # view when LW < 512), on the assigned engine
                        eng = assign[eidx % 20]
                        eidx += 1
                        if LW == 512:
                            src, dst = sc[:, :], exv[:, :]
                        else:
                            src = sc.rearrange("p (two l) -> p two l", two=2)[:, :, :LW]
                            dst = exv.rearrange("p (two l) -> p two l", two=2)[:, :, :LW]
                        if STAGE >= 2:
                            emit_exp(eng, src, dst)
                    else:
                        # half granularity: one sc bank + one exp per ci
                        for h in range(2):
                            ci = pair * 2 + h
                            sc = psA.tile([128, 512], FP, tag="sc", name="sc")
                            if STAGE >= 1:
                                nc.tensor.matmul(
                                    out=sc[:, :LW],
                                    lhsT=Hf8[:C, ci * 128 : (ci + 1) * 128],
                                    rhs=uwt8_v[:C, lb0 : lb0 + LW],
                                    start=True,
                                    stop=True,
                                )
                            eng = assign[eidx % 20]
                            eidx += 1
                            if STAGE >= 2:
                                emit_exp(eng, sc[:, :LW], exv[:, h * 512 : h * 512 + LW])
                    if pend is not None and STAGE >= 3:
                        emit_mm2(pend)
                    pend = (ma, ex, lb, LW, pair)
            if pend is not None and STAGE >= 3:
                emit_mm2(pend)
            pend = None

            ob = osball[:, b * LT : (b + 1) * LT]
            if STAGE < 5:
                nc.gpsimd.memset(ob, 0.0)
                continue
            # final: d = sum_c m*out_w; sigmoid((d/s) + bias)
            # prod/reduce split across Vector and GpSimd (label-tile halves)
            m3 = mlt.rearrange("p (t q) -> p t q", q=64)
            prod = workp.tile([128, LT * C], BF, tag="prod", name="prod")
            prod3 = prod.rearrange("p (t c) -> p t c", c=C)
            ow3 = owp_s.rearrange("p (t c) -> p t c", c=C)
            d = workp.tile([128, LT], FP, tag="d", name="d")
            HT = LT // 2
            nc.vector.tensor_mul(
                out=prod3[:, :HT], in0=m3[:, :HT, 0:C], in1=ow3[:, :HT]
            )
            nc.gpsimd.tensor_mul(
                out=prod3[:, HT:], in0=m3[:, HT:, 0:C], in1=ow3[:, HT:]
            )
            nc.vector.tensor_reduce(
                out=d[:, :HT],
                in_=prod3[:, :HT],
                axis=mybir.AxisListType.X,
                op=mybir.AluOpType.add,
            )
            nc.vector.tensor_reduce(
                out=d[:, HT:],
                in_=prod3[:, HT:],
                axis=mybir.AxisListType.X,
                op=mybir.AluOpType.add,
            )
            rs = workp.tile([128, LT], FP, tag="rs", name="rs")
            nc.vector.reciprocal(out=rs, in_=m3[:, :, C : C + 1])
            dz = workp.tile([128, LT], FP, tag="dz", name="dz")
            nc.gpsimd.tensor_mul(out=dz, in0=d, in1=rs)
            zt = workp.tile([128, LT], FP, tag="zt", name="zt")
            nc.gpsimd.tensor_add(out=zt, in0=dz, in1=obp_s)
            nc.scalar.activation(out=ob, in_=zt, func=AF.Sigmoid)

        # single batched output DMA for the whole body
        nc.sync.dma_start(
            out=out.rearrange("b p t -> p b t"),
            in_=osball.rearrange("p (b t) -> p b t", b=B),
        )


def host_prep(inputs):
    """Full inputs -> list of 8 per-core input maps."""
    import ml_dtypes

    f8np = mybir.dt.np(F8)

    x = np.asarray(inputs["x"]).astype(np.int32)
    wemb = np.ascontiguousarray(np.asarray(inputs["W_embed"], dtype=np.float32))
    conv_w = np.asarray(inputs["conv_w"], dtype=np.float32)
    conv_b = np.asarray(inputs["conv_b"], dtype=np.float32)
    u_w = np.asarray(inputs["u_w"], dtype=np.float32)
    out_w = np.asarray(inputs["out_w"], dtype=np.float32)
    out_b = np.asarray(inputs["out_b"], dtype=np.float32)

    xp = np.zeros((B, WPAD), np.int32)
    xp[:, :W] = x
    idx = np.ascontiguousarray(
        xp.reshape(B, NCI, 128).transpose(2, 0, 1).reshape(128, B * NCI)
    )
    convwt = np.ascontiguousarray(
        np.concatenate([conv_w[:, :, k].T for k in range(K)], axis=1)
    ).astype(ml_dtypes.bfloat16)  # (E, K*C)
    convbp = np.ascontiguousarray(conv_b.reshape(C, 1))

    shared = {"x_idx": idx, "convwt": convwt, "convb": convbp}
    if EMB_HOST:
        embt = np.zeros((E, B, 1032), np.float32)
        embt[:, :, 1 : 1 + W] = wemb[x].transpose(2, 0, 1)  # (E, B, W)
        shared["embt"] = np.ascontiguousarray(
            embt.reshape(E, B * 1032)
        ).astype(ml_dtypes.bfloat16)
        shared.pop("x_idx", None)
    else:
        shared["wemb"] = wemb
    in_maps = []
    for c in range(NCORES):
        l0 = c * LSH
        uw_pad = np.zeros((LPAD, C), np.float32)
        uw_pad[:LSH] = u_w[l0 : l0 + LSH]
        ow_pad = np.zeros((LPAD, C), np.float32)
        ow_pad[:LSH] = out_w[l0 : l0 + LSH]
        ob_pad = np.zeros(LPAD, np.float32)
        ob_pad[:LSH] = out_b[l0 : l0 + LSH]

        # mm1 rhs: plain fp8 u^T on partitions 0-49
        u8 = np.zeros((64, LPAD), f8np)
        u8[:C] = uw_pad.T.astype(f8np)

        in_maps.append(
            dict(
                shared,
                uwt8=np.ascontiguousarray(u8).view(np.uint8),
                owp=np.ascontiguousarray(
                    ow_pad.reshape(LT, 128, C).transpose(1, 0, 2).reshape(128, LT * C)
                ).astype(ml_dtypes.bfloat16),
                obp=np.ascontiguousarray(ob_pad.reshape(LT, 128).T),
            )
        )
    return in_maps


def unshard(outs):
    """outs: list of 8 arrays (B, 128, LT) -> (B, L)."""
    parts = [
        np.asarray(o).transpose(0, 2, 1).reshape(B, LPAD)[:, :LSH] for o in outs
    ]
    return np.ascontiguousarray(np.concatenate(parts, axis=1), dtype=np.float32)


_NC = None
LAST_RESULTS = None


def kernel(**inputs) -> np.ndarray:
    global _NC, LAST_RESULTS
    in_maps = host_prep(inputs)
    if _NC is None:
        _NC = build_nc(num_devices=NCORES)
    trace = bool(int(os.environ.get("KERNEL_TRACE", "0")))
    res = run_bass_kernel_spmd(
        _NC, in_maps, core_ids=list(range(NCORES)), trace=trace
    )
    LAST_RESULTS = res
    outs = [res.results[i]["out"] for i in range(NCORES)]
    return unshard(outs)



# revision 9
# speedup vs baseline: 4.8378x; 4.8378x over previous
"""CAML-style multi-label attention kernel for Trainium2 (8 NeuronCores).

Reference computation (B=8, W=1000, V=50000, E=100, C=50, K=3, L=18000):
    emb    = W_embed[x]                            (B, W, E)
    H      = tanh(conv1d(emb, conv_w) + conv_b)    (B, W, C)  'same' padding
    scores = einsum("lc,bwc->blw", u_w, H)
    attns  = softmax(scores, axis=w)
    m      = einsum("blw,bwc->blc", attns, H)
    out    = sigmoid(sum(out_w * m, axis=c) + out_b)   (B, L)

Sharding: L=18000 split across 8 cores (2250 labels each, padded to 2304).
The (tiny) conv prologue is replicated on every core.

Per-core algorithm (fp8e4m3 fast path; rel-err budget is 2e-2, this achieves
~4e-4 -- scores/weights are O(0.6) here so e4m3 quantization washes out
through the softmax):
  - host pre-gathers embedding rows into conv-rhs layout (embt, one DMA);
    conv = 3 accumulating bf16 matmuls; tanh writes H in fp8e4m3.
  - mm1 (scores): plain fp8 matmul, H stationary, u^T fp8 streaming.
  - exp via the Schraudolph bit trick: int8(SCHR_A*s + SCHR_B) reinterpreted
    as fp8e4m3 IS exp(s) (~7% pointwise, ~4e-4 after softmax averaging).
    A plain affine+convert, so it runs on BOTH ScalarE (activation Copy with
    scale/bias) and VectorE (tensor_scalar), splitting the 21M-element
    PSUM->SBUF exp pass across two engines. (GpSimd cannot access PSUM.)
  - mm2 (pooling): fp8 DoubleRow pairing adjacent 128-token chunks: lhsT is
    Haug8 [w128, (2, 64)] = H^T + ones column (-> softmax denominator) + pad
    (DoubleRow needs block stride %16==0); rhs is the ex tile [w128, (2,LW)]
    whose halves the exp stage already writes. fp8 PE transposes need
    element-step-2 output APs.
  - ma (PSUM) staged to SBUF bf16 (ScalarE/VectorE alternating), small PE
    transposes to label-partition layout, epilogue dot/divide on
    VectorE/GpSimd, final sigmoid on ScalarE.
  - per-batch prologue is software-pipelined one batch ahead of its main
    loop; DMA count per iteration is minimized (hoisted input loads, one
    batched output store) -- each in-body DMA costs several us of per-launch
    ring overhead on this hardware, dwarfing its nominal transfer time.
"""

import os

import numpy as np

try:
    import concourse.bass as bass
except ImportError:  # repo not on sys.path in fresh dirs
    import sys

    sys.path.insert(0, "/opt/trn_rl_repo")
    import concourse.bass as bass

import concourse.bacc as bacc
import concourse.tile as tile
from concourse import mybir
from concourse.bass import IndirectOffsetOnAxis
from concourse.bass_utils import run_bass_kernel_spmd
from concourse.masks import make_identity

FP = mybir.dt.float32
BF = mybir.dt.bfloat16
F8 = mybir.dt.float8e4
I8 = mybir.dt.int8
U8 = mybir.dt.uint8
AF = mybir.ActivationFunctionType
ALU = mybir.AluOpType
DR = mybir.MatmulPerfMode.DoubleRow

B, W, V, E, C, K, L = 8, 1000, 50000, 100, 50, 3, 18000
NCORES = 8
WPAD = 1024  # W padded to 8 chunks of 128
LSH = L // NCORES  # 2250 labels per core
LPAD = 2304  # 18 tiles of 128
LT = LPAD // 128  # 18 label tiles per core
NCI = WPAD // 128  # 8 w-chunks
CH = C // 2  # 25: contraction rows for DoubleRow mm1
C2 = 64  # Haug block width: H cols + ones col + zero pad (DoubleRow needs block stride %16==0)

# Schraudolph exp in fp8e4m3: exp(x) ~= bitcast_f8(int8(SCHR_A*x + SCHR_B))
SCHR_A = float(os.environ.get("SCHR_A", 8.0 / np.log(2.0)))
SCHR_B = float(os.environ.get("SCHR_B", "55.85"))

# exp engine split: counts out of 20 pair-chunks ([128, 1024]) per batch.
# GPSIMD cannot access PSUM on TRN2, so exp runs on ScalarE + VectorE only.
EXP_NA = int(os.environ.get("EXP_NA", "11"))  # ScalarE (activation Copy)
EXP_ND = int(os.environ.get("EXP_ND", "9"))  # VectorE (rest)

# ma PSUM->SBUF staging engine per lb-block (5 chars, A/D/P)
MSB_ENG = os.environ.get("MSB_ENG", "ADADA")

# 1: host pre-gathers embedding rows (embx input, DMA on SP queue);
# 0: device-side indirect gather on GpSimd
EMB_HOST = int(os.environ.get("EMB_HOST", "1"))

# HW ablation: 5=full, 4=no epilogue, 3=no post(msb/ptm/mlt), 2=no mm2,
# 1=no exp, 0=prologue only
STAGE = int(os.environ.get("STAGE", "5"))
EXPP_BUFS = int(os.environ.get("EXPP_BUFS", "4"))
# sc tiles are [128,1024] = 2 PSUM banks each now; 2 bufs + psB(2) + psT(1)
# + psP(1) fills all 8 banks
PSA_BUFS = int(os.environ.get("PSA_BUFS", "2"))
PSB_BUFS = int(os.environ.get("PSB_BUFS", "2"))
PST_BUFS = int(os.environ.get("PST_BUFS", "1"))


def _mix_assign(na, nd, np_, total=40):
    """Interleave engine assignments smoothly (weighted round-robin)."""
    quota = {"A": na, "D": nd, "P": np_}
    acc = {"A": 0.0, "D": 0.0, "P": 0.0}
    out = []
    for _ in range(total):
        for k in acc:
            acc[k] += quota[k] / total
        k = max(acc, key=lambda q: acc[q])
        acc[k] -= 1.0
        out.append(k)
    return out


def build_nc(num_devices: int, repeat: int = 1):
    nc = bacc.Bacc(
        "TRN2", target_bir_lowering=False, debug=False, num_devices=num_devices
    )

    if EMB_HOST:
        wemb = nc.dram_tensor("embt", [E, B * 1032], BF, kind="ExternalInput").ap()
    else:
        x_idx = nc.dram_tensor("x_idx", [128, B * NCI], mybir.dt.int32, kind="ExternalInput").ap()
        wemb = nc.dram_tensor("wemb", [V, E], FP, kind="ExternalInput").ap()
    convwt = nc.dram_tensor("convwt", [E, K * C], BF, kind="ExternalInput").ap()
    convb = nc.dram_tensor("convb", [C, 1], FP, kind="ExternalInput").ap()
    # u^T replicated at partitions 0-49 and 64-113 (row-tiled mm1 rhs)
    uwt8 = nc.dram_tensor("uwt8", [128, LPAD], U8, kind="ExternalInput").ap()
    # out_w^T on rows 0-49, ones on row 50 (denominator passthrough)
    owt = nc.dram_tensor("owt", [C + 1, LPAD], BF, kind="ExternalInput").ap()
    obp = nc.dram_tensor("obp", [128, LT], FP, kind="ExternalInput").ap()
    out = nc.dram_tensor("out", [B, 128, LT], FP, kind="ExternalOutput").ap()

    with tile.TileContext(nc) as tc:
        with tc.tile_pool(name="const", bufs=1) as constp:
            # all input loads happen ONCE per launch, outside the repeat
            # body (each DMA in the body costs ~7.5us of per-launch ring
            # overhead on HW, dwarfing the compute)
            cst = {}
            cst["ident"] = constp.tile([128, 128], FP, name="ident")
            make_identity(nc, cst["ident"])
            cst["ident_bf"] = constp.tile([128, 128], BF, name="ident_bf")
            make_identity(nc, cst["ident_bf"])
            cst["ident_f8"] = constp.tile([128, 128], F8, name="ident_f8")
            make_identity(nc, cst["ident_f8"])
            uwt8_s = constp.tile([128, LPAD], U8, name="uwt8_s")
            nc.sync.dma_start(out=uwt8_s, in_=uwt8)
            cst["uwt8_v"] = uwt8_s.bitcast(F8)
            cst["convwt_s"] = constp.tile([E, K * C], BF, name="convwt_s")
            nc.sync.dma_start(out=cst["convwt_s"], in_=convwt)
            cst["convb_s"] = constp.tile([C, 1], FP, name="convb_s")
            nc.sync.dma_start(out=cst["convb_s"], in_=convb)
            cst["owp_s"] = constp.tile([128, LT * C], BF, name="owp_s")
            nc.sync.dma_start(out=cst["owp_s"], in_=owp)
            cst["obp_s"] = constp.tile([128, LT], FP, name="obp_s")
            nc.sync.dma_start(out=cst["obp_s"], in_=obp)
            if EMB_HOST:
                cst["embt_s"] = constp.tile([E, B * 1032], BF, name="embt_s")
                nc.sync.dma_start(out=cst["embt_s"], in_=wemb)
                cst["idx_s"] = None
            else:
                cst["idx_s"] = constp.tile(
                    [128, B * NCI], mybir.dt.int32, name="idx_s"
                )
                nc.sync.dma_start(out=cst["idx_s"], in_=x_idx)
                cst["wemb"] = wemb
            for _ in range(repeat):
                _body(tc, nc, cst, out)
    nc.compile()
    return nc


def _body(tc, nc, cst, out):
    assign = _mix_assign(EXP_NA, EXP_ND, 20 - EXP_NA - EXP_ND, total=20)
    ident = cst["ident"]
    ident_bf = cst["ident_bf"]
    ident_f8 = cst["ident_f8"]
    uwt8_v = cst["uwt8_v"]
    convwt_s = cst["convwt_s"]
    convb_s = cst["convb_s"]
    owp_s = cst["owp_s"]
    obp_s = cst["obp_s"]
    with (
        tc.tile_pool(name="work", bufs=2) as workp,
        tc.tile_pool(name="expp", bufs=EXPP_BUFS) as expp,
        tc.tile_pool(name="psA", bufs=PSA_BUFS, space="PSUM") as psA,  # sc
        tc.tile_pool(name="psB", bufs=PSB_BUFS, space="PSUM") as psB,  # ma
        tc.tile_pool(name="psT", bufs=PST_BUFS, space="PSUM") as psT,  # ptm
        tc.tile_pool(name="psP", bufs=1, space="PSUM") as psP,  # prologue
        tc.tile_pool(name="outp", bufs=2) as outp,
    ):
        def gather_emb(b):
            # non-host path: 8 indirect gathers on the GpSimd SWDGE queue
            embG = workp.tile([128, NCI * E], BF, tag="embG", name="embG", bufs=3)
            for ci in range(NCI):
                nc.gpsimd.indirect_dma_start(
                    out=embG[:, ci * E : (ci + 1) * E],
                    out_offset=None,
                    in_=cst["wemb"][:, :],
                    in_offset=IndirectOffsetOnAxis(
                        ap=cst["idx_s"][:, b * NCI + ci : b * NCI + ci + 1], axis=0
                    ),
                )
            return embG

        def prologue(b, embG_b):
            # embP -> conv -> tanh -> H tiles (fp8). Issued one batch AHEAD
            # of its main loop so its engine slots sit in front of the
            # previous batch's exp backlog in the queues.
            # Hf8 layout: w-half h (512 cols) lives at partitions 64h..64h+C,
            # so mm1 can run the two halves' chunks CONCURRENTLY as row-tiles
            # (0,0)/(64,0) of the PE array.
            Hf8 = workp.tile([128, WPAD // 2], F8, tag="Hf8", name="Hf8")
            Haug8 = workp.tile([128, NCI * C2], F8, tag="Haug8", name="Haug8")
            if EMB_HOST:
                embP = cst["embt_s"][:, b * 1032 : (b + 1) * 1032]
            else:
                embP = workp.tile([E, 1032], BF, tag="embP", name="embP")
                nc.gpsimd.memset(embP[:, 0:1], 0.0)
                nc.gpsimd.memset(embP[:, 1001:1032], 0.0)
                for ci in range(NCI):
                    pt = psP.tile([128, 128], BF, tag="pp", name="pt")
                    nc.tensor.transpose(
                        out=pt[:E, :],
                        in_=embG_b[:, ci * E : (ci + 1) * E],
                        identity=ident_bf[:, :],
                    )
                    cw = min(128, W - ci * 128)
                    nc.vector.tensor_copy(
                        out=embP[:, 1 + ci * 128 : 1 + ci * 128 + cw], in_=pt[:E, :cw]
                    )

            # conv1d col-tiled: the two w-halves output to partitions {0, 64}
            # of ONE PSUM bank and run concurrently (PE col strips 0-1 / 2-3)
            pm = psP.tile([128, 512], FP, tag="pp", name="convps")
            for half, (w0, cw) in enumerate(((0, 512), (512, W - 512))):
                pb = 64 * half
                for k in range(K):
                    nc.tensor.matmul(
                        out=pm[pb : pb + C, :cw],
                        lhsT=convwt_s[:, k * C : (k + 1) * C],
                        rhs=embP[:, w0 + k : w0 + k + cw],
                        start=(k == 0),
                        stop=(k == K - 1),
                        tile_position=(0, pb),
                    )
                nc.scalar.activation(
                    out=Hf8[pb : pb + C, :cw],
                    in_=pm[pb : pb + C, :cw],
                    func=AF.Tanh,
                    bias=convb_s[:, 0:1],
                )
            nc.gpsimd.memset(Hf8[64 : 64 + C, W - 512 : 512], 0.0)

            # Haug8 block order pairs (g, g+4) adjacently so mm2's DoubleRow
            # lhsT slice matches the (half0, half1) ex pairing of mm1.
            for ci in range(NCI):
                half, col = divmod(ci, 4)
                pb = 64 * half
                pt2 = psP.tile([128, 256], F8, tag="pp", name="pt2")
                # fp8 transpose requires output element step 2 (16-bit PE
                # datapath); write strided, then the pack copy reads strided
                pt2v = pt2.rearrange("p (c two) -> p two c", two=2)
                nc.tensor.transpose(
                    out=pt2v[:, 0, :C],
                    in_=Hf8[pb : pb + C, col * 128 : (col + 1) * 128],
                    identity=ident_f8[pb : pb + C, pb : pb + C],
                )
                base = ((ci % 4) * 2 + (ci // 4)) * C2
                nc.vector.tensor_copy(
                    out=Haug8[:, base : base + C], in_=pt2v[:, 0, :C]
                )
                nc.gpsimd.memset(Haug8[:, base + C + 1 : base + C2], 0.0)
                if ci < NCI - 1:
                    nc.gpsimd.memset(Haug8[:, base + C : base + C + 1], 1.0)
                else:
                    nc.gpsimd.memset(Haug8[:, base + C : base + C + 1], 0.0)
                    nc.gpsimd.memset(Haug8[: W - 896, base + C : base + C + 1], 1.0)
            return Hf8, Haug8

        # batched output staging: one DMA per body instead of one per batch
        osball = outp.tile([128, B * LT], FP, tag="osball", name="osball")

        # pipeline fill: prologue leads the main loop by 1 batch
        if EMB_HOST:
            H_cur = prologue(0, None)
            embG_next = None
        else:
            embG_next = gather_emb(0)
            H_cur = prologue(0, embG_next)
            if B > 1:
                embG_next = gather_emb(1)

        for b in range(B):
            Hf8, Haug8 = H_cur
            if b + 1 < B:
                H_cur = prologue(b + 1, embG_next)
            if not EMB_HOST and b + 2 < B:
                embG_next = gather_emb(b + 2)

            # ------------- main: label blocks for this batch ----------------
            # per-batch label-partition results: [p, lt*64 + (0..49 m, 50 s)]
            mlt = workp.tile([128, LT * 64], BF, tag="mlt", name="mlt")

            def lb_post(ma_u, lb_u, LW_u):
                # PSUM -> SBUF staging (to bf16, engine tunable), then small
                # PE transposes back to label-partition layout, packed
                # 4-per-bank so one strided 2x-mode copy moves them all
                msb = workp.tile([C + 1, 512], BF, tag="msb", name="msb")
                meng = MSB_ENG[lb_u % len(MSB_ENG)]
                if meng == "A":
                    nc.scalar.activation(
                        out=msb[:, :LW_u], in_=ma_u[: C + 1, :LW_u], func=AF.Copy
                    )
                else:
                    nc.vector.tensor_copy(out=msb[:, :LW_u], in_=ma_u[: C + 1, :LW_u])
                nq = LW_u // 128
                ptm = psT.tile([128, 256], BF, tag="pt", name="ptm")
                for q in range(nq):
                    nc.tensor.transpose(
                        out=ptm[:, q * 64 : q * 64 + C + 1],
                        in_=msb[:, q * 128 : (q + 1) * 128],
                        identity=ident_bf[: C + 1, : C + 1],
                    )
                nc.vector.tensor_copy(
                    out=mlt.rearrange("p (t s) -> p t s", s=64)[
                        :, lb_u * 4 : lb_u * 4 + nq, 0 : C + 1
                    ],
                    in_=ptm.rearrange("p (q s) -> p q s", s=64)[:, 0:nq, 0 : C + 1],
                )

            def emit_mm2(u):
                ma_u, ex_u, lb_u, LW_u, pair_u = u
                lhsT = Haug8[
                    :, pair_u * 2 * C2 : (pair_u * 2 + 2) * C2
                ].rearrange("p (two f) -> p two f", two=2)
                rhs = ex_u.bitcast(F8).rearrange("p (two l) -> p two l", two=2)[
                    :, :, :LW_u
                ]
                nc.tensor.matmul(
                    out=ma_u[:, :LW_u],
                    lhsT=lhsT,
                    rhs=rhs,
                    start=(pair_u == 0),
                    stop=(pair_u == 3),
                    perf_mode=DR,
                )
                if pair_u == 3 and STAGE >= 4:
                    lb_post(ma_u, lb_u, LW_u)

            def emit_exp(eng, src, dst):
                if eng == "A":
                    nc.scalar.activation(
                        out=dst, in_=src, func=AF.Copy,
                        bias=SCHR_B, scale=SCHR_A,
                    )
                else:
                    nc.vector.tensor_scalar(
                        out=dst, in0=src, scalar1=SCHR_A,
                        scalar2=SCHR_B, op0=ALU.mult, op1=ALU.add,
                    )

            # one-stage software pipeline: emit mm2 of the previous pair after
            # this pair's mm1+exp, so PE is never queued behind a stalled mm2.
            # mm1 pair g = chunks (g, g+4): the two matmuls run CONCURRENTLY
            # as PE row-tiles (0,0)/(64,0) (K=50 occupies only rows 0-63 /
            # 64-127), writing the two banks of one [128,1024] sc tile that a
            # single FD=1024 exp instruction then drains.
            pend = None
            eidx = 0
            for lb in range(5):
                lb0 = lb * 512
                LW = min(512, LPAD - lb0)
                ma = psB.tile([C2, 512], FP, tag="maug", name="ma")
                for g in range(4):
                    ex = expp.tile([128, 1024], U8, tag="ex", name="ex")
                    exv = ex.bitcast(I8)
                    sc = psA.tile([128, 1024], FP, tag="sc", name="sc")
                    for half in range(2):
                        pb = 64 * half
                        if STAGE >= 1:
                            nc.tensor.matmul(
                                out=sc[:, half * 512 : half * 512 + LW],
                                lhsT=Hf8[pb : pb + C, g * 128 : (g + 1) * 128],
                                rhs=uwt8_v[pb : pb + C, lb0 : lb0 + LW],
                                start=True,
                                stop=True,
                                tile_position=(pb, 0),
                            )
                    # one Schraudolph-exp instruction per pair (strided
                    # view when LW < 512), on the assigned engine
                    eng = assign[eidx % 20]
                    eidx += 1
                    if LW == 512:
                        src, dst = sc[:, :], exv[:, :]
                    else:
                        src = sc.rearrange("p (two l) -> p two l", two=2)[:, :, :LW]
                        dst = exv.rearrange("p (two l) -> p two l", two=2)[:, :, :LW]
                    if STAGE >= 2:
                        emit_exp(eng, src, dst)
                    if pend is not None and STAGE >= 3:
                        emit_mm2(pend)
                    pend = (ma, ex, lb, LW, g)
            if pend is not None and STAGE >= 3:
                emit_mm2(pend)
            pend = None

            ob = osball[:, b * LT : (b + 1) * LT]
            if STAGE < 5:
                nc.gpsimd.memset(ob, 0.0)
                continue
            # final: d = sum_c m*out_w; sigmoid((d/s) + bias)
            # prod/reduce split across Vector and GpSimd (label-tile halves)
            m3 = mlt.rearrange("p (t q) -> p t q", q=64)
            prod = workp.tile([128, LT * C], BF, tag="prod", name="prod")
            prod3 = prod.rearrange("p (t c) -> p t c", c=C)
            ow3 = owp_s.rearrange("p (t c) -> p t c", c=C)
            d = workp.tile([128, LT], FP, tag="d", name="d")
            HT = LT // 2
            nc.vector.tensor_mul(
                out=prod3[:, :HT], in0=m3[:, :HT, 0:C], in1=ow3[:, :HT]
            )
            nc.gpsimd.tensor_mul(
                out=prod3[:, HT:], in0=m3[:, HT:, 0:C], in1=ow3[:, HT:]
            )
            nc.vector.tensor_reduce(
                out=d[:, :HT],
                in_=prod3[:, :HT],
                axis=mybir.AxisListType.X,
                op=mybir.AluOpType.add,
            )
            nc.vector.tensor_reduce(
                out=d[:, HT:],
                in_=prod3[:, HT:],
                axis=mybir.AxisListType.X,
                op=mybir.AluOpType.add,
            )
            rs = workp.tile([128, LT], FP, tag="rs", name="rs")
            nc.vector.reciprocal(out=rs, in_=m3[:, :, C : C + 1])
            dz = workp.tile([128, LT], FP, tag="dz", name="dz")
            nc.gpsimd.tensor_mul(out=dz, in0=d, in1=rs)
            zt = workp.tile([128, LT], FP, tag="zt", name="zt")
            nc.gpsimd.tensor_add(out=zt, in0=dz, in1=obp_s)
            nc.scalar.activation(out=ob, in_=zt, func=AF.Sigmoid)

        # single batched output DMA for the whole body
        nc.sync.dma_start(
            out=out.rearrange("b p t -> p b t"),
            in_=osball.rearrange("p (b t) -> p b t", b=B),
        )


def host_prep(inputs):
    """Full inputs -> list of 8 per-core input maps."""
    import ml_dtypes

    f8np = mybir.dt.np(F8)

    x = np.asarray(inputs["x"]).astype(np.int32)
    wemb = np.ascontiguousarray(np.asarray(inputs["W_embed"], dtype=np.float32))
    conv_w = np.asarray(inputs["conv_w"], dtype=np.float32)
    conv_b = np.asarray(inputs["conv_b"], dtype=np.float32)
    u_w = np.asarray(inputs["u_w"], dtype=np.float32)
    out_w = np.asarray(inputs["out_w"], dtype=np.float32)
    out_b = np.asarray(inputs["out_b"], dtype=np.float32)

    xp = np.zeros((B, WPAD), np.int32)
    xp[:, :W] = x
    idx = np.ascontiguousarray(
        xp.reshape(B, NCI, 128).transpose(2, 0, 1).reshape(128, B * NCI)
    )
    convwt = np.ascontiguousarray(
        np.concatenate([conv_w[:, :, k].T for k in range(K)], axis=1)
    ).astype(ml_dtypes.bfloat16)  # (E, K*C)
    convbp = np.ascontiguousarray(conv_b.reshape(C, 1))

    shared = {"x_idx": idx, "convwt": convwt, "convb": convbp}
    if EMB_HOST:
        embt = np.zeros((E, B, 1032), np.float32)
        embt[:, :, 1 : 1 + W] = wemb[x].transpose(2, 0, 1)  # (E, B, W)
        shared["embt"] = np.ascontiguousarray(
            embt.reshape(E, B * 1032)
        ).astype(ml_dtypes.bfloat16)
        shared.pop("x_idx", None)
    else:
        shared["wemb"] = wemb
    in_maps = []
    for c in range(NCORES):
        l0 = c * LSH
        uw_pad = np.zeros((LPAD, C), np.float32)
        uw_pad[:LSH] = u_w[l0 : l0 + LSH]
        ow_pad = np.zeros((LPAD, C), np.float32)
        ow_pad[:LSH] = out_w[l0 : l0 + LSH]
        ob_pad = np.zeros(LPAD, np.float32)
        ob_pad[:LSH] = out_b[l0 : l0 + LSH]

        # mm1 rhs: plain fp8 u^T, replicated at partitions 0-49 and 64-113
        # (one copy per mm1 row-tile)
        u8 = np.zeros((128, LPAD), f8np)
        u8[:C] = uw_pad.T.astype(f8np)
        u8[64 : 64 + C] = u8[:C]

        in_maps.append(
            dict(
                shared,
                uwt8=np.ascontiguousarray(u8).view(np.uint8),
                owp=np.ascontiguousarray(
                    ow_pad.reshape(LT, 128, C).transpose(1, 0, 2).reshape(128, LT * C)
                ).astype(ml_dtypes.bfloat16),
                obp=np.ascontiguousarray(ob_pad.reshape(LT, 128).T),
            )
        )
    return in_maps


def unshard(outs):
    """outs: list of 8 arrays (B, 128, LT) -> (B, L)."""
    parts = [
        np.asarray(o).transpose(0, 2, 1).reshape(B, LPAD)[:, :LSH] for o in outs
    ]
    return np.ascontiguousarray(np.concatenate(parts, axis=1), dtype=np.float32)


_NC = None
LAST_RESULTS = None


def kernel(**inputs) -> np.ndarray:
    global _NC, LAST_RESULTS
    in_maps = host_prep(inputs)
    if _NC is None:
        _NC = build_nc(num_devices=NCORES)
    trace = bool(int(os.environ.get("KERNEL_TRACE", "0")))
    res = run_bass_kernel_spmd(
        _NC, in_maps, core_ids=list(range(NCORES)), trace=trace
    )
    LAST_RESULTS = res
    outs = [res.results[i]["out"] for i in range(NCORES)]
    return unshard(outs)

